# revision 24
# baseline (speedup 1.0000x reference)
"""Trainium2 Bass kernel: per-pixel 5x5-patch channel covariance.

R[b,h,w,k,l] = (1/N) sum_n (p_kn - mu_k)(p_ln - mu_l)   (N=25, reflect pad)

Identity:  R = box5x5(S_k * S_l)/25 - mu_k * mu_l,  mu = box5x5(S)/25.
Separable box sums run as banded matmuls on TensorE; reflect padding is
folded into the band weights. Host pre-scales S by 1/5 so the two band
passes produce box/25 directly.

v3: pair products are computed ONCE on a 128-row tile (plus a folded
32-partition tile for the 4 tail rows, reshaped by DMA into the 4-row
lhsT the tail matmuls need), instead of twice on 68-row tiles. The H-box
runs as 3 matmuls per (channel, w-half): rows 0-63 from the head band,
rows 64-127 from the mid band, plus a 4-wide tail accumulation. Only the
136 upper-triangle pair channels are computed/DMA'd (pair-major); the
host mirrors to the full 16x16. Work is split across DVE / Act / GpSimd.

Sharding: 8 cores = 4 batches x 2 H-halves. Fully data parallel.
"""
import sys

sys.path.insert(0, "/opt/trn_rl_repo")

from contextlib import ExitStack

import numpy as np

import concourse.bacc as bacc
import concourse.mybir as mybir
import concourse.tile as tile
from concourse import bass_utils

B, K, H, W = 4, 16, 256, 256
HH = 128           # output rows per core
SR = 132           # shard rows (128 + 2 halo each side, edge-clamped)
NP = 136           # upper-triangle pairs, k-major: (0,0)..(0,15),(1,1)..
NCH = K + NP       # 152 channels: 0..15 mean, 16.. pairs (pr order)
NOCT = NCH // 8    # 19 channel octets (oct 0,1 = means; 2..18 = pairs)
NL0 = 72           # pairs in first L tile (octs 2..10); rest in second
F32 = mybir.dt.float32
BF16 = mybir.dt.bfloat16

# ---- tuning knobs (engine routing) ----
D_POOL_FRAC = 3    # every Nth sub unit routed Act-evac + GpSimd-sub
C_POOL_FRAC = 3    # every Nth M-octet built on GpSimd instead of DVE
B_DVE_FRAC = 4     # every Nth stage-1 evac copied by DVE instead of Act


def _reflect_idx(i, n):
    if i < 0:
        return -i
    if i >= n:
        return 2 * (n - 1) - i
    return i


def _build_bw():
    """[256 w'col, 256 wout] box weights with reflection folded; -> [128, 4*128]
    blocks indexed (oh, chunk): BW[:, (oh*2+c)*128 + wl] = M[c*128 + :, oh*128 + wl]."""
    M = np.zeros((W, W), dtype=np.float32)
    for w in range(W):
        for j in range(5):
            M[_reflect_idx(w - 2 + j, W), w] += 1.0
    out = np.zeros((128, 512), dtype=np.float32)
    for oh in range(2):
        for c in range(2):
            out[:, (oh * 2 + c) * 128:(oh * 2 + c) * 128 + 128] = \
                M[c * 128:(c + 1) * 128, oh * 128:(oh + 1) * 128]
    return out


def _build_brp(half):
    """H-box band, reflect folded, two partition-aligned packs:
    BR  [68, 68]: cols 0:64 head (out rows 0..63 from shard rows 0..67),
                  cols 64:68 tail (out rows 124..127 from shard rows 128..131,
                  band at partitions 0..3)
    BR2 [128, 64]: mid (out rows 64..127 from shard rows 64..127, band
                  stored at partitions 64..127)."""
    hbase = half * HH
    M132 = np.zeros((SR, HH), dtype=np.float32)
    for r in range(HH):
        for i in range(5):
            g = _reflect_idx(hbase + r - 2 + i, H)
            s = g - (hbase - 2)
            M132[s, r] += 1.0
    br = np.zeros((68, 68), dtype=np.float32)
    br[0:68, 0:64] = M132[0:68, 0:64]
    br[0:4, 64:68] = M132[128:132, 124:128]
    br2 = np.zeros((128, 64), dtype=np.float32)
    br2[64:128, :] = M132[64:128, 64:128]
    # coverage check: nothing outside the three packed blocks
    chk = M132.copy()
    chk[0:68, 0:64] = 0
    chk[64:128, 64:128] = 0
    chk[128:132, 124:128] = 0
    assert not chk.any(), "band pack dropped nonzero entries"
    return br, br2


def _ksegs_in_octet(oct_idx):
    """Pair channels live at ch 16..151 (pr k-major). For octet [oct*8, +8),
    return (j0, k, l0, nl): local offset j0, channel k, first l, count."""
    lo, hi = oct_idx * 8, oct_idx * 8 + 8
    segs = []
    p = 0
    for k in range(K):
        n = K - k
        s, e = 16 + p, 16 + p + n
        a, b = max(lo, s), min(hi, e)
        if a < b:
            segs.append((a - lo, k, k + (a - s), b - a))
        p += n
    return segs


def _pr0(k):
    """pr index of pair (k, k)."""
    return k * K - (k * (k - 1)) // 2


def _build_kernel():
    nc = bacc.Bacc("TRN2", target_bir_lowering=False, debug=False)
    SM_d = nc.dram_tensor("SM", [128, K, W], BF16, kind="ExternalInput").ap()
    ST_d = nc.dram_tensor("ST", [4, K, W], BF16, kind="ExternalInput").ap()
    SF_d = nc.dram_tensor("SF", [32, K, 32], BF16, kind="ExternalInput").ap()
    BR_d = nc.dram_tensor("BR", [68, 68], BF16, kind="ExternalInput").ap()
    BR2_d = nc.dram_tensor("BR2", [128, 64], BF16, kind="ExternalInput").ap()
    BW_d = nc.dram_tensor("BW", [128, 512], BF16, kind="ExternalInput").ap()
    # output: upper triangle only, [w, rt, pr, hh] bf16
    R_d = nc.dram_tensor("R", [W, 2, NP, 64], BF16, kind="ExternalOutput").ap()

    with tile.TileContext(nc) as tc, ExitStack() as ctx:
        const_p = ctx.enter_context(tc.tile_pool(name="const", bufs=1))
        sp_p = ctx.enter_context(tc.tile_pool(name="sp", bufs=1))
        tf_p = ctx.enter_context(tc.tile_pool(name="tf", bufs=1))
        L_p = ctx.enter_context(tc.tile_pool(name="L", bufs=1))
        t_p = ctx.enter_context(tc.tile_pool(name="tprod", bufs=3))
        i1_p = ctx.enter_context(tc.tile_pool(name="i1", bufs=4))
        mu_p = ctx.enter_context(tc.tile_pool(name="mu", bufs=1))
        m_p = ctx.enter_context(tc.tile_pool(name="mm", bufs=3))
        r_p = ctx.enter_context(tc.tile_pool(name="rout", bufs=1))
        e2_p = ctx.enter_context(tc.tile_pool(name="e2", bufs=2))
        ps1_p = ctx.enter_context(tc.tile_pool(name="ps1", bufs=2, space="PSUM"))
        ps2_p = ctx.enter_context(tc.tile_pool(name="ps2", bufs=4, space="PSUM"))

        brp = const_p.tile([68, 68], BF16)
        brp2 = const_p.tile([128, 64], BF16)
        bw = const_p.tile([128, 512], BF16)
        nc.sync.dma_start(brp[:], BR_d)
        nc.sync.dma_start(brp2[:], BR2_d)
        nc.sync.dma_start(bw[:], BW_d)

        sm = sp_p.tile([128, K, W], BF16)
        st = sp_p.tile([4, K, W], BF16)
        sf = sp_p.tile([32, K, 32], BF16)
        nc.sync.dma_start(sm[:], SM_d)
        nc.sync.dma_start(st[:], ST_d)
        nc.sync.dma_start(sf[:], SF_d)

        # ---- tail products (folded 32-partition tile), reshaped into L ----
        tf = tf_p.tile([32, NP, 32], BF16)
        for k in range(K):
            nl = K - k
            p0 = _pr0(k)
            in0 = sf[:, k, :].unsqueeze(1).broadcast_to([32, nl, 32])
            nc.vector.tensor_mul(tf[:, p0:p0 + nl, :], in0, sf[:, k:K, :])

        L = L_p.tile([4, NL0, W], BF16, name="L")
        for b in range(8):
            nc.sync.dma_start(L[:, :, b * 32:(b + 1) * 32],
                              tf[b * 4:(b + 1) * 4, 0:NL0, :])

        bra = brp[0:68, 0:64]
        brb = brp2[64:128, :]
        brc = brp[0:4, 64:68]
        bwh = [(bw[:, (oh * 2) * 128:(oh * 2) * 128 + 128],
                bw[:, (oh * 2 + 1) * 128:(oh * 2 + 1) * 128 + 128])
               for oh in range(2)]

        # mub_all[:, rt*2+oh, k, :]; rsb_all[:, rt, oh, pr, :]
        mub_all = mu_p.tile([128, 4, K, 64], BF16, name="mub")
        rsb_all = r_p.tile([128, 2, 2, NP, 64], BF16, name="rsb")

        pending_evacs = []

        def stage1(oc, L, L2, defer_dve_evac=True):
            """products (pair octs) + H-box into a fresh per-octet i1 tile.
            DVE-routed psum evacs are deferred to the next iteration via
            pending_evacs to keep the in-order DVE stream stall-free."""
            if oc < 2:   # mean channels read straight from sm / st
                T = sm[:, oc * 8:(oc + 1) * 8, :]
                TL = st[:, oc * 8:(oc + 1) * 8, :]
            else:
                Tt = t_p.tile([128, 8, W], BF16, name="T")
                for (j0, k, l0, nl) in _ksegs_in_octet(oc):
                    in0 = sm[:, k, :].unsqueeze(1).broadcast_to([128, nl, W])
                    nc.vector.tensor_mul(
                        Tt[:, j0:j0 + nl, :], in0, sm[:, l0:l0 + nl, :])
                T = Tt[:]
                pr = (oc - 2) * 8
                if pr < NL0:
                    TL = L[:, pr:pr + 8, :]
                else:
                    TL = L2[:, pr - NL0:pr - NL0 + 8, :]
            # i1oc layout: [w(128), rt(2), wchunk(2), ch(8), h(64)]
            i1oc = i1_p.tile([128, 2, 2, 8, 64], BF16, name="i1")
            for wh in range(2):
                ps1 = ps1_p.tile([128, 2, 8, 64], F32, name="ps1")
                ws = wh * 128
                for j in range(8):
                    nc.tensor.matmul(ps1[:, 0, j, :],
                                     T[0:68, j, ws:ws + 128], bra,
                                     start=True, stop=True)
                    nc.tensor.matmul(ps1[:, 1, j, :],
                                     T[64:128, j, ws:ws + 128], brb,
                                     start=True, stop=False,
                                     skip_group_check=True)
                    nc.tensor.matmul(ps1[:, 1, j, 60:64],
                                     TL[0:4, j, ws:ws + 128], brc,
                                     start=False, stop=True,
                                     skip_group_check=True)
                if defer_dve_evac and oc >= 2:
                    on_dve = (2 * oc + wh) % B_DVE_FRAC == 1
                    pending_evacs.append((i1oc, wh, ps1, on_dve))
                else:
                    nc.scalar.copy(i1oc[:, :, wh], ps1[:])
            return i1oc

        def flush_evacs():
            while pending_evacs:
                i1oc, wh, ps1, on_dve = pending_evacs.pop(0)
                if on_dve:
                    nc.vector.tensor_copy(i1oc[:, :, wh], ps1[:])
                else:
                    nc.scalar.copy(i1oc[:, :, wh], ps1[:])

        def p2(i1oc, oc, rt, oh):
            (bwa, bwb) = bwh[oh]
            ps2 = ps2_p.tile([128, 8, 64], F32, name="ps2")
            nc.tensor.matmul(ps2[:], bwa, i1oc[:, rt, 0],
                             start=True, stop=False)
            nc.tensor.matmul(ps2[:], bwb, i1oc[:, rt, 1],
                             start=False, stop=True)
            return ps2

        # ---- mean octets -> mub ----
        i1m = [stage1(oc, None, None) for oc in range(2)]
        for rt in range(2):
            for oh in range(2):
                for mo in range(2):
                    psm = p2(i1m[mo], mo, rt, oh)
                    nc.scalar.mul(mub_all[:, rt * 2 + oh, mo * 8:(mo + 1) * 8],
                                  psm[:], 0.2)

        def stage2_finish(ps2s, oc):
            """M builds + subtracts for octet oc (e2 copies emitted first so
            the Act stream isn't blocked behind this iteration's evacs)."""
            pr = (oc - 2) * 8
            plan = []
            for ps2, rt, oh in ps2s:
                gunit = (oc - 2) * 4 + rt * 2 + oh
                if gunit % D_POOL_FRAC == D_POOL_FRAC - 1:
                    e2 = e2_p.tile([128, 8, 64], BF16, name="e2")
                    nc.scalar.copy(e2[:], ps2[:])
                    plan.append((e2, rt, oh, gunit, True))
                else:
                    plan.append((ps2, rt, oh, gunit, False))
            for src, rt, oh, gunit, on_pool in plan:
                mub = mub_all[:, rt * 2 + oh]
                M = m_p.tile([128, 8, 64], BF16, name="M")
                meng = nc.gpsimd if gunit % C_POOL_FRAC == 1 else nc.vector
                for (j0, k, l0, nl) in _ksegs_in_octet(oc):
                    in0 = mub[:, k, :].unsqueeze(1).broadcast_to(
                        [128, nl, 64])
                    meng.tensor_mul(M[:, j0:j0 + nl, :], in0,
                                    mub[:, l0:l0 + nl, :])
                dst = rsb_all[:, rt, oh, pr:pr + 8, :]
                if on_pool:
                    nc.gpsimd.tensor_sub(dst, src[:], M[:])
                else:
                    nc.vector.tensor_sub(dst, src[:], M[:])

        # ---- pair octets: software pipelined, stage2 lags LAG octets ----
        LAG = 2
        L2 = None
        i1q = {}    # oc -> i1 tile
        for it in range(2, NOCT + LAG):
            oc2 = it - LAG     # stage2 octet this iteration
            # 1) P2 matmuls first: deps settled long ago, keeps PE fed
            ps2s = None
            if oc2 >= 2:
                i1oc = i1q.pop(oc2)
                ps2s = [(p2(i1oc, oc2, rt, oh), rt, oh)
                        for rt in range(2) for oh in range(2)]
            # 2) next octet's products + P1 (DVE gets ready work first)
            if it < NOCT:
                i1q[it] = stage1(it, L, L2)
                if it == 10:   # L fully consumed; refill remaining pairs
                    L2 = L_p.tile([4, NL0, W], BF16, name="L")
                    for b in range(8):
                        nc.sync.dma_start(
                            L2[:, 0:NP - NL0, b * 32:(b + 1) * 32],
                            tf[b * 4:(b + 1) * 4, NL0:NP, :])
            # 3) stage2 finishers (M + sub), then 4) deferred evacs
            if ps2s is not None:
                stage2_finish(ps2s, oc2)
            flush_evacs()

        for rt in range(2):
            for oh in range(2):
                nc.sync.dma_start(R_d[oh * 128:(oh + 1) * 128, rt],
                                  rsb_all[:, rt, oh])

    nc.compile()
    return nc


_NC_CACHE = {}


def _get_nc():
    if "nc" not in _NC_CACHE:
        _NC_CACHE["nc"] = _build_kernel()
    return _NC_CACHE["nc"]


def _prep_in_maps(S):
    S = np.asarray(S, dtype=np.float32)
    np_bf16 = mybir.dt.np(BF16)
    bw = _build_bw().astype(np_bf16)
    brs = [tuple(x.astype(np_bf16) for x in _build_brp(h)) for h in range(2)]
    Ss = S * np.float32(0.2)
    in_maps = []
    for b in range(B):
        for half in range(2):
            hbase = half * HH
            rows = np.clip(np.arange(hbase - 2, hbase + 130), 0, H - 1)
            shard = Ss[b][:, rows, :].transpose(1, 0, 2)   # [132, K, 256]
            shard = np.ascontiguousarray(shard).astype(np_bf16)
            sm = shard[0:128]
            stail = np.ascontiguousarray(shard[128:132])   # [4, K, 256]
            # fold: SF[wdiv*4 + r', c, j] = ST[r', c, wdiv*32 + j]
            sfold = np.ascontiguousarray(
                stail.reshape(4, K, 8, 32).transpose(2, 0, 1, 3)
                .reshape(32, K, 32))
            in_maps.append({"SM": sm, "ST": stail, "SF": sfold,
                            "BR": brs[half][0], "BR2": brs[half][1],
                            "BW": bw})
    return in_maps


# upper-tri gather index: IU[k, l] = pr for (min,max)
_IU = np.zeros((K, K), dtype=np.int64)
for _k in range(K):
    for _l in range(_k, K):
        _IU[_k, _l] = _IU[_l, _k] = _pr0(_k) + (_l - _k)


def _assemble(results):
    out = np.empty((B, H, W, K, K), dtype=np.float32)
    for i in range(8):
        b, half = divmod(i, 2)
        rd = np.asarray(results[i]["R"]).astype(np.float32)  # [256, 2, 136, 64]
        tri = rd.transpose(1, 3, 0, 2).reshape(HH, W, NP)    # [h, w, pr]
        out[b, half * HH:(half + 1) * HH] = tri[:, :, _IU]
    return out


def kernel(S):
    """S: [4, 16, 256, 256] float32 -> R: [4, 256, 256, 16, 16] float32."""
    nc = _get_nc()
    in_maps = _prep_in_maps(S)
    res = bass_utils.run_bass_kernel_spmd(nc, in_maps, list(range(8)))
    return _assemble(res.results)


# revision 28
# speedup vs baseline: 1.2116x; 1.2116x over previous
"""2a' structure (per-rt phases) with engine-routing knobs.

R = box5x5(S_k*S_l)/25 - mu_k*mu_l; banded matmuls on TensorE; triangle
output; per-rt stage1/stage2 phases (rt-level overlap)."""
import sys

sys.path.insert(0, "/opt/trn_rl_repo")

from contextlib import ExitStack

import numpy as np

import concourse.bacc as bacc
import concourse.mybir as mybir
import concourse.tile as tile
from concourse import bass_utils

B, K, H, W = 4, 16, 256, 256
HH = 128
SR = 132
NP = 136
NCH = K + NP
NOCT = NCH // 8
F32 = mybir.dt.float32
BF16 = mybir.dt.bfloat16

D_POOL_FRAC = 3    # every Nth sub unit routed Act-evac + GpSimd-sub
C_POOL_KMAX = 3    # M k-runs with k < KMAX on GpSimd
A_POOL_KMAX = 0    # product k-runs with k < KMAX on GpSimd


def _reflect_idx(i, n):
    if i < 0:
        return -i
    if i >= n:
        return 2 * (n - 1) - i
    return i


def _build_bw():
    M = np.zeros((W, W), dtype=np.float32)
    for w in range(W):
        for j in range(5):
            M[_reflect_idx(w - 2 + j, W), w] += 1.0
    out = np.zeros((128, 512), dtype=np.float32)
    for oh in range(2):
        for c in range(2):
            out[:, (oh * 2 + c) * 128:(oh * 2 + c) * 128 + 128] = \
                M[c * 128:(c + 1) * 128, oh * 128:(oh + 1) * 128]
    return out


def _build_br(half):
    hbase = half * HH
    M = np.zeros((68, 128), dtype=np.float32)
    for rt in range(2):
        for hl in range(64):
            hg = hbase + rt * 64 + hl
            for i in range(5):
                r = _reflect_idx(hg - 2 + i, H)
                j = r + 2 - hbase
                M[j - rt * 64, rt * 64 + hl] += 1.0
    return M


def _ksegs_in_octet(oct_idx):
    lo, hi = oct_idx * 8, oct_idx * 8 + 8
    segs = []
    p = 0
    for k in range(K):
        n = K - k
        s, e = 16 + p, 16 + p + n
        a, b = max(lo, s), min(hi, e)
        if a < b:
            segs.append((a - lo, k, k + (a - s), b - a))
        p += n
    return segs


def _pr0(k):
    return k * K - (k * (k - 1)) // 2


def _build_kernel():
    nc = bacc.Bacc("TRN2", target_bir_lowering=False, debug=False)
    S_d = nc.dram_tensor("S", [SR, K, W], BF16, kind="ExternalInput").ap()
    BR_d = nc.dram_tensor("BR", [68, 128], BF16, kind="ExternalInput").ap()
    BW_d = nc.dram_tensor("BW", [128, 512], BF16, kind="ExternalInput").ap()
    R_d = nc.dram_tensor("R", [W, 2, NP, 64], BF16, kind="ExternalOutput").ap()

    with tile.TileContext(nc) as tc, ExitStack() as ctx:
        const_p = ctx.enter_context(tc.tile_pool(name="const", bufs=1))
        sp_p = ctx.enter_context(tc.tile_pool(name="sp", bufs=1))
        t_p = ctx.enter_context(tc.tile_pool(name="tprod", bufs=3))
        i1_p = ctx.enter_context(tc.tile_pool(name="i1", bufs=2))
        mu_p = ctx.enter_context(tc.tile_pool(name="mu", bufs=2))
        m_p = ctx.enter_context(tc.tile_pool(name="mm", bufs=2))
        r_p = ctx.enter_context(tc.tile_pool(name="rout", bufs=2))
        e2_p = ctx.enter_context(tc.tile_pool(name="e2", bufs=3))
        ps1_p = ctx.enter_context(tc.tile_pool(name="ps1", bufs=2, space="PSUM"))
        ps2_p = ctx.enter_context(tc.tile_pool(name="ps2", bufs=4, space="PSUM"))

        br = const_p.tile([68, 128], BF16)
        bw = const_p.tile([128, 512], BF16)
        nc.sync.dma_start(br[:], BR_d)
        nc.sync.dma_start(bw[:], BW_d)

        sp0 = sp_p.tile([68, K, W], BF16)
        sp1 = sp_p.tile([68, K, W], BF16)
        nc.sync.dma_start(sp0[:], S_d[0:68])
        nc.sync.dma_start(sp1[:], S_d[64:132])
        sps = [sp0, sp1]

        for rt in range(2):
            sp = sps[rt]
            brt = br[:, rt * 64:(rt + 1) * 64]
            # ---- stage 1: products + H-box ----
            i1 = i1_p.tile([128, 2, NCH, 64], BF16, name="i1")
            for oc in range(NOCT):
                if oc < 2:
                    T = sp[:, oc * 8:(oc + 1) * 8, :]
                else:
                    Tt = t_p.tile([68, 8, W], BF16, name="T")
                    for (j0, k, l0, nl) in _ksegs_in_octet(oc):
                        in0 = sp[:, k, :].unsqueeze(1).broadcast_to([68, nl, W])
                        eng = nc.gpsimd if k < A_POOL_KMAX else nc.vector
                        eng.tensor_mul(
                            Tt[:, j0:j0 + nl, :], in0, sp[:, l0:l0 + nl, :])
                    T = Tt[:]
                ps1 = ps1_p.tile([128, 2, 8, 64], F32, name="ps1")
                for j in range(8):
                    nc.tensor.matmul(ps1[:, 0, j, :],
                                     T[0:68, j, 0:128], brt,
                                     start=True, stop=True)
                    nc.tensor.matmul(ps1[:, 1, j, :],
                                     T[0:68, j, 128:256], brt,
                                     start=True, stop=True)
                nc.scalar.copy(i1[:, :, oc * 8:(oc + 1) * 8, :], ps1[:])

            # ---- stage 2: W-box + finish ----
            for oh in range(2):
                bwa = bw[:, (oh * 2) * 128:(oh * 2) * 128 + 128]
                bwb = bw[:, (oh * 2 + 1) * 128:(oh * 2 + 1) * 128 + 128]
                mub = mu_p.tile([128, K, 64], BF16, name="mub")
                for mo in range(2):
                    psm = ps2_p.tile([128, 8, 64], F32, name="ps2")
                    nc.tensor.matmul(psm[:], bwa,
                                     i1[:, 0, mo * 8:(mo + 1) * 8, :],
                                     start=True, stop=False)
                    nc.tensor.matmul(psm[:], bwb,
                                     i1[:, 1, mo * 8:(mo + 1) * 8, :],
                                     start=False, stop=True)
                    nc.scalar.mul(mub[:, mo * 8:(mo + 1) * 8, :], psm[:], 0.2)
                M = m_p.tile([128, NP, 64], BF16, name="M")
                for k in range(K):
                    nl = K - k
                    p0 = _pr0(k)
                    in0 = mub[:, k, :].unsqueeze(1).broadcast_to([128, nl, 64])
                    eng = nc.gpsimd if k < C_POOL_KMAX else nc.vector
                    eng.tensor_mul(M[:, p0:p0 + nl, :], in0, mub[:, k:K, :])
                rsb = r_p.tile([128, NP, 64], BF16, name="rsb")
                for oc in range(2, NOCT):
                    ps2 = ps2_p.tile([128, 8, 64], F32, name="ps2")
                    nc.tensor.matmul(ps2[:], bwa,
                                     i1[:, 0, oc * 8:(oc + 1) * 8, :],
                                     start=True, stop=False)
                    nc.tensor.matmul(ps2[:], bwb,
                                     i1[:, 1, oc * 8:(oc + 1) * 8, :],
                                     start=False, stop=True)
                    pr = (oc - 2) * 8
                    gunit = (rt * 2 + oh) * 17 + (oc - 2)
                    if gunit % D_POOL_FRAC == D_POOL_FRAC - 1:
                        e2 = e2_p.tile([128, 8, 64], BF16, name="e2")
                        nc.scalar.copy(e2[:], ps2[:])
                        nc.gpsimd.tensor_sub(rsb[:, pr:pr + 8, :], e2[:],
                                             M[:, pr:pr + 8, :])
                    else:
                        nc.vector.tensor_sub(rsb[:, pr:pr + 8, :], ps2[:],
                                             M[:, pr:pr + 8, :])
                nc.sync.dma_start(R_d[oh * 128:(oh + 1) * 128, rt], rsb[:])

    nc.compile()
    return nc


_NC_CACHE = {}


def _get_nc():
    if "nc" not in _NC_CACHE:
        _NC_CACHE["nc"] = _build_kernel()
    return _NC_CACHE["nc"]


def _prep_in_maps(S):
    S = np.asarray(S, dtype=np.float32)
    np_bf16 = mybir.dt.np(BF16)
    bw = _build_bw().astype(np_bf16)
    brs = [(_build_br(h)).astype(np_bf16) for h in range(2)]
    Ss = S * np.float32(0.2)
    in_maps = []
    for b in range(B):
        for half in range(2):
            hbase = half * HH
            rows = np.clip(np.arange(hbase - 2, hbase + 130), 0, H - 1)
            shard = Ss[b][:, rows, :].transpose(1, 0, 2)
            shard = np.ascontiguousarray(shard).astype(np_bf16)
            in_maps.append({"S": shard, "BR": brs[half], "BW": bw})
    return in_maps


_IU = np.zeros((K, K), dtype=np.int64)
for _k in range(K):
    for _l in range(_k, K):
        _IU[_k, _l] = _IU[_l, _k] = _pr0(_k) + (_l - _k)


def _assemble(results):
    out = np.empty((B, H, W, K, K), dtype=np.float32)
    for i in range(8):
        b, half = divmod(i, 2)
        rd = np.asarray(results[i]["R"]).astype(np.float32)
        tri = rd.transpose(1, 3, 0, 2).reshape(HH, W, NP)
        out[b, half * HH:(half + 1) * HH] = tri[:, :, _IU]
    return out


def kernel(S):
    nc = _get_nc()
    in_maps = _prep_in_maps(S)
    res = bass_utils.run_bass_kernel_spmd(nc, in_maps, list(range(8)))
    return _assemble(res.results)


# revision 30
# speedup vs baseline: 1.2280x; 1.0135x over previous
"""2a' structure (per-rt phases) with engine-routing knobs.

R = box5x5(S_k*S_l)/25 - mu_k*mu_l; banded matmuls on TensorE; triangle
output; per-rt stage1/stage2 phases (rt-level overlap)."""
import sys

sys.path.insert(0, "/opt/trn_rl_repo")

from contextlib import ExitStack

import numpy as np

import concourse.bacc as bacc
import concourse.mybir as mybir
import concourse.tile as tile
from concourse import bass_utils

B, K, H, W = 4, 16, 256, 256
HH = 128
SR = 132
NP = 136
NCH = K + NP
NOCT = NCH // 8
F32 = mybir.dt.float32
BF16 = mybir.dt.bfloat16

D_POOL_FRAC = 3    # every Nth sub unit routed Act-evac + GpSimd-sub
T_BUFS = 4
E2_BUFS = 4
M_BUFS = 3
PS1_BUFS = 2
PS2_BUFS = 4
C_POOL_KMAX = 3    # M k-runs with k < KMAX on GpSimd
A_POOL_KMAX = 0    # product k-runs with k < KMAX on GpSimd


def _reflect_idx(i, n):
    if i < 0:
        return -i
    if i >= n:
        return 2 * (n - 1) - i
    return i


def _build_bw():
    M = np.zeros((W, W), dtype=np.float32)
    for w in range(W):
        for j in range(5):
            M[_reflect_idx(w - 2 + j, W), w] += 1.0
    out = np.zeros((128, 512), dtype=np.float32)
    for oh in range(2):
        for c in range(2):
            out[:, (oh * 2 + c) * 128:(oh * 2 + c) * 128 + 128] = \
                M[c * 128:(c + 1) * 128, oh * 128:(oh + 1) * 128]
    return out


def _build_br(half):
    hbase = half * HH
    M = np.zeros((68, 128), dtype=np.float32)
    for rt in range(2):
        for hl in range(64):
            hg = hbase + rt * 64 + hl
            for i in range(5):
                r = _reflect_idx(hg - 2 + i, H)
                j = r + 2 - hbase
                M[j - rt * 64, rt * 64 + hl] += 1.0
    return M


def _ksegs_in_octet(oct_idx):
    lo, hi = oct_idx * 8, oct_idx * 8 + 8
    segs = []
    p = 0
    for k in range(K):
        n = K - k
        s, e = 16 + p, 16 + p + n
        a, b = max(lo, s), min(hi, e)
        if a < b:
            segs.append((a - lo, k, k + (a - s), b - a))
        p += n
    return segs


def _pr0(k):
    return k * K - (k * (k - 1)) // 2


def _build_kernel():
    nc = bacc.Bacc("TRN2", target_bir_lowering=False, debug=False)
    S_d = nc.dram_tensor("S", [SR, K, W], BF16, kind="ExternalInput").ap()
    BR_d = nc.dram_tensor("BR", [68, 128], BF16, kind="ExternalInput").ap()
    BW_d = nc.dram_tensor("BW", [128, 512], BF16, kind="ExternalInput").ap()
    R_d = nc.dram_tensor("R", [W, 2, NP, 64], BF16, kind="ExternalOutput").ap()

    with tile.TileContext(nc) as tc, ExitStack() as ctx:
        const_p = ctx.enter_context(tc.tile_pool(name="const", bufs=1))
        sp_p = ctx.enter_context(tc.tile_pool(name="sp", bufs=1))
        t_p = ctx.enter_context(tc.tile_pool(name="tprod", bufs=T_BUFS))
        i1_p = ctx.enter_context(tc.tile_pool(name="i1", bufs=2))
        mu_p = ctx.enter_context(tc.tile_pool(name="mu", bufs=2))
        m_p = ctx.enter_context(tc.tile_pool(name="mm", bufs=M_BUFS))
        r_p = ctx.enter_context(tc.tile_pool(name="rout", bufs=2))
        e2_p = ctx.enter_context(tc.tile_pool(name="e2", bufs=E2_BUFS))
        ps1_p = ctx.enter_context(tc.tile_pool(name="ps1", bufs=PS1_BUFS, space="PSUM"))
        ps2_p = ctx.enter_context(tc.tile_pool(name="ps2", bufs=PS2_BUFS, space="PSUM"))

        br = const_p.tile([68, 128], BF16)
        bw = const_p.tile([128, 512], BF16)
        nc.sync.dma_start(br[:], BR_d)
        nc.sync.dma_start(bw[:], BW_d)

        sp0 = sp_p.tile([68, K, W], BF16)
        sp1 = sp_p.tile([68, K, W], BF16)
        nc.sync.dma_start(sp0[:], S_d[0:68])
        nc.sync.dma_start(sp1[:], S_d[64:132])
        sps = [sp0, sp1]

        for rt in range(2):
            sp = sps[rt]
            brt = br[:, rt * 64:(rt + 1) * 64]
            # ---- stage 1: products + H-box ----
            i1 = i1_p.tile([128, 2, NCH, 64], BF16, name="i1")
            for oc in range(NOCT):
                if oc < 2:
                    T = sp[:, oc * 8:(oc + 1) * 8, :]
                else:
                    Tt = t_p.tile([68, 8, W], BF16, name="T")
                    for (j0, k, l0, nl) in _ksegs_in_octet(oc):
                        in0 = sp[:, k, :].unsqueeze(1).broadcast_to([68, nl, W])
                        eng = nc.gpsimd if k < A_POOL_KMAX else nc.vector
                        eng.tensor_mul(
                            Tt[:, j0:j0 + nl, :], in0, sp[:, l0:l0 + nl, :])
                    T = Tt[:]
                ps1 = ps1_p.tile([128, 2, 8, 64], F32, name="ps1")
                for j in range(8):
                    nc.tensor.matmul(ps1[:, 0, j, :],
                                     T[0:68, j, 0:128], brt,
                                     start=True, stop=True)
                    nc.tensor.matmul(ps1[:, 1, j, :],
                                     T[0:68, j, 128:256], brt,
                                     start=True, stop=True)
                nc.scalar.copy(i1[:, :, oc * 8:(oc + 1) * 8, :], ps1[:])

            # ---- stage 2: W-box + finish ----
            for oh in range(2):
                bwa = bw[:, (oh * 2) * 128:(oh * 2) * 128 + 128]
                bwb = bw[:, (oh * 2 + 1) * 128:(oh * 2 + 1) * 128 + 128]
                mub = mu_p.tile([128, K, 64], BF16, name="mub")
                for mo in range(2):
                    psm = ps2_p.tile([128, 8, 64], F32, name="ps2")
                    nc.tensor.matmul(psm[:], bwa,
                                     i1[:, 0, mo * 8:(mo + 1) * 8, :],
                                     start=True, stop=False)
                    nc.tensor.matmul(psm[:], bwb,
                                     i1[:, 1, mo * 8:(mo + 1) * 8, :],
                                     start=False, stop=True)
                    nc.scalar.mul(mub[:, mo * 8:(mo + 1) * 8, :], psm[:], 0.2)
                M = m_p.tile([128, NP, 64], BF16, name="M")
                for k in range(K):
                    nl = K - k
                    p0 = _pr0(k)
                    in0 = mub[:, k, :].unsqueeze(1).broadcast_to([128, nl, 64])
                    eng = nc.gpsimd if k < C_POOL_KMAX else nc.vector
                    eng.tensor_mul(M[:, p0:p0 + nl, :], in0, mub[:, k:K, :])
                rsb = r_p.tile([128, NP, 64], BF16, name="rsb")
                for oc in range(2, NOCT):
                    ps2 = ps2_p.tile([128, 8, 64], F32, name="ps2")
                    nc.tensor.matmul(ps2[:], bwa,
                                     i1[:, 0, oc * 8:(oc + 1) * 8, :],
                                     start=True, stop=False)
                    nc.tensor.matmul(ps2[:], bwb,
                                     i1[:, 1, oc * 8:(oc + 1) * 8, :],
                                     start=False, stop=True)
                    pr = (oc - 2) * 8
                    gunit = (rt * 2 + oh) * 17 + (oc - 2)
                    if gunit % D_POOL_FRAC == D_POOL_FRAC - 1:
                        e2 = e2_p.tile([128, 8, 64], BF16, name="e2")
                        nc.scalar.copy(e2[:], ps2[:])
                        nc.gpsimd.tensor_sub(rsb[:, pr:pr + 8, :], e2[:],
                                             M[:, pr:pr + 8, :])
                    else:
                        nc.vector.tensor_sub(rsb[:, pr:pr + 8, :], ps2[:],
                                             M[:, pr:pr + 8, :])
                nc.sync.dma_start(R_d[oh * 128:(oh + 1) * 128, rt], rsb[:])

    nc.compile()
    return nc


_NC_CACHE = {}


def _get_nc():
    if "nc" not in _NC_CACHE:
        _NC_CACHE["nc"] = _build_kernel()
    return _NC_CACHE["nc"]


def _prep_in_maps(S):
    S = np.asarray(S, dtype=np.float32)
    np_bf16 = mybir.dt.np(BF16)
    bw = _build_bw().astype(np_bf16)
    brs = [(_build_br(h)).astype(np_bf16) for h in range(2)]
    Ss = S * np.float32(0.2)
    in_maps = []
    for b in range(B):
        for half in range(2):
            hbase = half * HH
            rows = np.clip(np.arange(hbase - 2, hbase + 130), 0, H - 1)
            shard = Ss[b][:, rows, :].transpose(1, 0, 2)
            shard = np.ascontiguousarray(shard).astype(np_bf16)
            in_maps.append({"S": shard, "BR": brs[half], "BW": bw})
    return in_maps


_IU = np.zeros((K, K), dtype=np.int64)
for _k in range(K):
    for _l in range(_k, K):
        _IU[_k, _l] = _IU[_l, _k] = _pr0(_k) + (_l - _k)


def _assemble(results):
    out = np.empty((B, H, W, K, K), dtype=np.float32)
    for i in range(8):
        b, half = divmod(i, 2)
        rd = np.asarray(results[i]["R"]).astype(np.float32)
        tri = rd.transpose(1, 3, 0, 2).reshape(HH, W, NP)
        out[b, half * HH:(half + 1) * HH] = tri[:, :, _IU]
    return out


def kernel(S):
    nc = _get_nc()
    in_maps = _prep_in_maps(S)
    res = bass_utils.run_bass_kernel_spmd(nc, in_maps, list(range(8)))
    return _assemble(res.results)


# revision 37
# speedup vs baseline: 1.4249x; 1.1603x over previous
"""2a' structure (per-rt phases) with engine-routing knobs.

R = box5x5(S_k*S_l)/25 - mu_k*mu_l; banded matmuls on TensorE; triangle
output; per-rt stage1/stage2 phases (rt-level overlap)."""
import sys

sys.path.insert(0, "/opt/trn_rl_repo")

from contextlib import ExitStack

import numpy as np

import concourse.bacc as bacc
import concourse.mybir as mybir
import concourse.tile as tile
from concourse import bass_utils

B, K, H, W = 4, 16, 256, 256
HH = 128
SR = 132
NP = 136
NCH = K + NP
NOCT = NCH // 8
F32 = mybir.dt.float32
BF16 = mybir.dt.bfloat16

D_POOL_FRAC = 3    # every Nth sub unit routed Act-evac + GpSimd-sub
T_BUFS = 4
E2_BUFS = 4
M_BUFS = 3
PS1_BUFS = 2
PS2_BUFS = 4
C_POOL_KMAX = 7    # M k-runs with k < KMAX on GpSimd
A_POOL_KMAX = 0    # product k-runs with k < KMAX on GpSimd
# stream output DMA: emit partial stores after these octets
DMA_SPLIT_AT = (4, 8, 12, 15, 17)
_splits = (2,) + tuple(o + 1 for o in DMA_SPLIT_AT)
DMA_LO = {o: (_splits[i] - 2) * 8 for i, o in enumerate(DMA_SPLIT_AT)}
DMA_TAIL_LO = (_splits[-1] - 2) * 8


def _reflect_idx(i, n):
    if i < 0:
        return -i
    if i >= n:
        return 2 * (n - 1) - i
    return i


def _build_bw():
    M = np.zeros((W, W), dtype=np.float32)
    for w in range(W):
        for j in range(5):
            M[_reflect_idx(w - 2 + j, W), w] += 1.0
    out = np.zeros((128, 512), dtype=np.float32)
    for oh in range(2):
        for c in range(2):
            out[:, (oh * 2 + c) * 128:(oh * 2 + c) * 128 + 128] = \
                M[c * 128:(c + 1) * 128, oh * 128:(oh + 1) * 128]
    return out


def _build_br(half):
    hbase = half * HH
    M = np.zeros((68, 128), dtype=np.float32)
    for rt in range(2):
        for hl in range(64):
            hg = hbase + rt * 64 + hl
            for i in range(5):
                r = _reflect_idx(hg - 2 + i, H)
                j = r + 2 - hbase
                M[j - rt * 64, rt * 64 + hl] += 1.0
    return M


def _ksegs_in_octet(oct_idx):
    lo, hi = oct_idx * 8, oct_idx * 8 + 8
    segs = []
    p = 0
    for k in range(K):
        n = K - k
        s, e = 16 + p, 16 + p + n
        a, b = max(lo, s), min(hi, e)
        if a < b:
            segs.append((a - lo, k, k + (a - s), b - a))
        p += n
    return segs


def _pr0(k):
    return k * K - (k * (k - 1)) // 2


def _build_kernel():
    nc = bacc.Bacc("TRN2", target_bir_lowering=False, debug=False)
    S_d = nc.dram_tensor("S", [SR, K, W], BF16, kind="ExternalInput").ap()
    BR_d = nc.dram_tensor("BR", [68, 128], BF16, kind="ExternalInput").ap()
    BW_d = nc.dram_tensor("BW", [128, 512], BF16, kind="ExternalInput").ap()
    R_d = nc.dram_tensor("R", [W, 2, NP, 64], BF16, kind="ExternalOutput").ap()

    with tile.TileContext(nc) as tc, ExitStack() as ctx:
        const_p = ctx.enter_context(tc.tile_pool(name="const", bufs=1))
        sp_p = ctx.enter_context(tc.tile_pool(name="sp", bufs=1))
        t_p = ctx.enter_context(tc.tile_pool(name="tprod", bufs=T_BUFS))
        i1_p = ctx.enter_context(tc.tile_pool(name="i1", bufs=2))
        mu_p = ctx.enter_context(tc.tile_pool(name="mu", bufs=2))
        m_p = ctx.enter_context(tc.tile_pool(name="mm", bufs=M_BUFS))
        r_p = ctx.enter_context(tc.tile_pool(name="rout", bufs=2))
        e2_p = ctx.enter_context(tc.tile_pool(name="e2", bufs=E2_BUFS))
        ps1_p = ctx.enter_context(tc.tile_pool(name="ps1", bufs=PS1_BUFS, space="PSUM"))
        ps2_p = ctx.enter_context(tc.tile_pool(name="ps2", bufs=PS2_BUFS, space="PSUM"))

        br = const_p.tile([68, 128], BF16)
        bw = const_p.tile([128, 512], BF16)
        nc.sync.dma_start(br[:], BR_d)
        nc.sync.dma_start(bw[:], BW_d)

        sp0 = sp_p.tile([68, K, W], BF16)
        sp1 = sp_p.tile([68, K, W], BF16)
        nc.sync.dma_start(sp0[:], S_d[0:68])
        nc.sync.dma_start(sp1[:], S_d[64:132])
        sps = [sp0, sp1]

        for rt in range(2):
            sp = sps[rt]
            brt = br[:, rt * 64:(rt + 1) * 64]
            Ms = []
            # ---- stage 1: products + H-box ----
            i1 = i1_p.tile([128, 2, NCH, 64], BF16, name="i1")
            for oc in range(NOCT):
                if oc < 2:
                    T = sp[:, oc * 8:(oc + 1) * 8, :]
                else:
                    Tt = t_p.tile([68, 8, W], BF16, name="T")
                    for (j0, k, l0, nl) in _ksegs_in_octet(oc):
                        in0 = sp[:, k, :].unsqueeze(1).broadcast_to([68, nl, W])
                        eng = nc.gpsimd if k < A_POOL_KMAX else nc.vector
                        eng.tensor_mul(
                            Tt[:, j0:j0 + nl, :], in0, sp[:, l0:l0 + nl, :])
                    T = Tt[:]
                ps1 = ps1_p.tile([128, 2, 8, 64], F32, name="ps1")
                for j in range(8):
                    nc.tensor.matmul(ps1[:, 0, j, :],
                                     T[0:68, j, 0:128], brt,
                                     start=True, stop=True)
                    nc.tensor.matmul(ps1[:, 1, j, :],
                                     T[0:68, j, 128:256], brt,
                                     start=True, stop=True)
                nc.scalar.copy(i1[:, :, oc * 8:(oc + 1) * 8, :], ps1[:])
                if oc == 1:
                    # hoist mean W-box + mu: Act/PE handle it while DVE is
                    # still busy with products, unblocking M-builds early
                    mubs = []
                    for oh in range(2):
                        bwa = bw[:, (oh * 2) * 128:(oh * 2) * 128 + 128]
                        bwb = bw[:, (oh * 2 + 1) * 128:(oh * 2 + 1) * 128 + 128]
                        mub = mu_p.tile([128, K, 64], BF16, name="mub")
                        mubs.append(mub)
                        for mo in range(2):
                            psm = ps2_p.tile([128, 8, 64], F32, name="ps2")
                            nc.tensor.matmul(psm[:], bwa,
                                             i1[:, 0, mo * 8:(mo + 1) * 8, :],
                                             start=True, stop=False)
                            nc.tensor.matmul(psm[:], bwb,
                                             i1[:, 1, mo * 8:(mo + 1) * 8, :],
                                             start=False, stop=True)
                            nc.scalar.mul(mub[:, mo * 8:(mo + 1) * 8, :],
                                          psm[:], 0.2)
                if oc in (4, 6):
                    # hoist M = mu_k*mu_l build between product octets
                    oh = 0 if oc == 4 else 1
                    mub = mubs[oh]
                    M = m_p.tile([128, NP, 64], BF16, name="M")
                    Ms.append(M)
                    for k in range(K):
                        nl = K - k
                        p0 = _pr0(k)
                        in0 = mub[:, k, :].unsqueeze(1).broadcast_to(
                            [128, nl, 64])
                        eng = nc.gpsimd if k < C_POOL_KMAX else nc.vector
                        eng.tensor_mul(M[:, p0:p0 + nl, :], in0,
                                       mub[:, k:K, :])

            # ---- stage 2: W-box + finish ----
            for oh in range(2):
                bwa = bw[:, (oh * 2) * 128:(oh * 2) * 128 + 128]
                bwb = bw[:, (oh * 2 + 1) * 128:(oh * 2 + 1) * 128 + 128]
                M = Ms[oh]
                rsb = r_p.tile([128, NP, 64], BF16, name="rsb")
                for oc in range(2, NOCT):
                    ps2 = ps2_p.tile([128, 8, 64], F32, name="ps2")
                    nc.tensor.matmul(ps2[:], bwa,
                                     i1[:, 0, oc * 8:(oc + 1) * 8, :],
                                     start=True, stop=False)
                    nc.tensor.matmul(ps2[:], bwb,
                                     i1[:, 1, oc * 8:(oc + 1) * 8, :],
                                     start=False, stop=True)
                    pr = (oc - 2) * 8
                    gunit = (rt * 2 + oh) * 17 + (oc - 2)
                    if gunit % D_POOL_FRAC == D_POOL_FRAC - 1:
                        e2 = e2_p.tile([128, 8, 64], BF16, name="e2")
                        nc.scalar.copy(e2[:], ps2[:])
                        nc.gpsimd.tensor_sub(rsb[:, pr:pr + 8, :], e2[:],
                                             M[:, pr:pr + 8, :])
                    else:
                        nc.vector.tensor_sub(rsb[:, pr:pr + 8, :], ps2[:],
                                             M[:, pr:pr + 8, :])
                    # stream the store out in chunks so the final DMA only
                    # covers the tail octets instead of the whole triangle
                    if oc in DMA_SPLIT_AT:
                        lo = DMA_LO[oc]
                        nc.sync.dma_start(
                            R_d[oh * 128:(oh + 1) * 128, rt, lo:pr + 8, :],
                            rsb[:, lo:pr + 8, :])
                nc.sync.dma_start(
                    R_d[oh * 128:(oh + 1) * 128, rt, DMA_TAIL_LO:, :],
                    rsb[:, DMA_TAIL_LO:, :])

    nc.compile()
    return nc


_NC_CACHE = {}


def _get_nc():
    if "nc" not in _NC_CACHE:
        _NC_CACHE["nc"] = _build_kernel()
    return _NC_CACHE["nc"]


def _prep_in_maps(S):
    S = np.asarray(S, dtype=np.float32)
    np_bf16 = mybir.dt.np(BF16)
    bw = _build_bw().astype(np_bf16)
    brs = [(_build_br(h)).astype(np_bf16) for h in range(2)]
    Ss = S * np.float32(0.2)
    in_maps = []
    for b in range(B):
        for half in range(2):
            hbase = half * HH
            rows = np.clip(np.arange(hbase - 2, hbase + 130), 0, H - 1)
            shard = Ss[b][:, rows, :].transpose(1, 0, 2)
            shard = np.ascontiguousarray(shard).astype(np_bf16)
            in_maps.append({"S": shard, "BR": brs[half], "BW": bw})
    return in_maps


_IU = np.zeros((K, K), dtype=np.int64)
for _k in range(K):
    for _l in range(_k, K):
        _IU[_k, _l] = _IU[_l, _k] = _pr0(_k) + (_l - _k)


def _assemble(results):
    out = np.empty((B, H, W, K, K), dtype=np.float32)
    for i in range(8):
        b, half = divmod(i, 2)
        rd = np.asarray(results[i]["R"]).astype(np.float32)
        tri = rd.transpose(1, 3, 0, 2).reshape(HH, W, NP)
        out[b, half * HH:(half + 1) * HH] = tri[:, :, _IU]
    return out


def kernel(S):
    nc = _get_nc()
    in_maps = _prep_in_maps(S)
    res = bass_utils.run_bass_kernel_spmd(nc, in_maps, list(range(8)))
    return _assemble(res.results)


# revision 43
# speedup vs baseline: 1.4459x; 1.0148x over previous
"""2a' structure (per-rt phases) with engine-routing knobs.

R = box5x5(S_k*S_l)/25 - mu_k*mu_l; banded matmuls on TensorE; triangle
output; per-rt stage1/stage2 phases (rt-level overlap)."""
import sys

sys.path.insert(0, "/opt/trn_rl_repo")

from contextlib import ExitStack

import numpy as np

import concourse.bacc as bacc
import concourse.mybir as mybir
import concourse.tile as tile
from concourse import bass_utils

B, K, H, W = 4, 16, 256, 256
HH = 128
SR = 132
NP = 136
NCH = K + NP
NOCT = NCH // 8
F32 = mybir.dt.float32
BF16 = mybir.dt.bfloat16

D_POOL_FRAC = (4, 3, 3, 3)   # per (rt*2+oh) phase: every Nth sub on GpSimd
T_BUFS = 4
E2_BUFS = 4
M_BUFS = 3
PS1_BUFS = 2
PS2_BUFS = 4
C_POOL_KMAX = 7    # M k-runs with k < KMAX on GpSimd
A_POOL_KMAX = 0    # product k-runs with k < KMAX on GpSimd
S1_SPLIT = 0       # rt1 stage-1 octets emitted before stage2(rt0, oh1)
# stream output DMA: emit partial stores after these octets
DMA_SPLIT_AT = (4, 8, 12, 15, 17)
_splits = (2,) + tuple(o + 1 for o in DMA_SPLIT_AT)
DMA_LO = {o: (_splits[i] - 2) * 8 for i, o in enumerate(DMA_SPLIT_AT)}
DMA_TAIL_LO = (_splits[-1] - 2) * 8


def _reflect_idx(i, n):
    if i < 0:
        return -i
    if i >= n:
        return 2 * (n - 1) - i
    return i


def _build_bw():
    M = np.zeros((W, W), dtype=np.float32)
    for w in range(W):
        for j in range(5):
            M[_reflect_idx(w - 2 + j, W), w] += 1.0
    out = np.zeros((128, 512), dtype=np.float32)
    for oh in range(2):
        for c in range(2):
            out[:, (oh * 2 + c) * 128:(oh * 2 + c) * 128 + 128] = \
                M[c * 128:(c + 1) * 128, oh * 128:(oh + 1) * 128]
    return out


def _build_br(half):
    hbase = half * HH
    M = np.zeros((68, 128), dtype=np.float32)
    for rt in range(2):
        for hl in range(64):
            hg = hbase + rt * 64 + hl
            for i in range(5):
                r = _reflect_idx(hg - 2 + i, H)
                j = r + 2 - hbase
                M[j - rt * 64, rt * 64 + hl] += 1.0
    return M


def _ksegs_in_octet(oct_idx):
    lo, hi = oct_idx * 8, oct_idx * 8 + 8
    segs = []
    p = 0
    for k in range(K):
        n = K - k
        s, e = 16 + p, 16 + p + n
        a, b = max(lo, s), min(hi, e)
        if a < b:
            segs.append((a - lo, k, k + (a - s), b - a))
        p += n
    return segs


def _pr0(k):
    return k * K - (k * (k - 1)) // 2


def _build_kernel():
    nc = bacc.Bacc("TRN2", target_bir_lowering=False, debug=False)
    S_d = nc.dram_tensor("S", [SR, K, W], BF16, kind="ExternalInput").ap()
    BR_d = nc.dram_tensor("BR", [68, 128], BF16, kind="ExternalInput").ap()
    BW_d = nc.dram_tensor("BW", [128, 512], BF16, kind="ExternalInput").ap()
    R_d = nc.dram_tensor("R", [W, 2, NP, 64], BF16, kind="ExternalOutput").ap()

    with tile.TileContext(nc) as tc, ExitStack() as ctx:
        const_p = ctx.enter_context(tc.tile_pool(name="const", bufs=1))
        sp_p = ctx.enter_context(tc.tile_pool(name="sp", bufs=1))
        t_p = ctx.enter_context(tc.tile_pool(name="tprod", bufs=T_BUFS))
        i1_p = ctx.enter_context(tc.tile_pool(name="i1", bufs=2))
        mu_p = ctx.enter_context(tc.tile_pool(name="mu", bufs=2))
        m_p = ctx.enter_context(tc.tile_pool(name="mm", bufs=M_BUFS))
        r_p = ctx.enter_context(tc.tile_pool(name="rout", bufs=2))
        e2_p = ctx.enter_context(tc.tile_pool(name="e2", bufs=E2_BUFS))
        ps1_p = ctx.enter_context(tc.tile_pool(name="ps1", bufs=PS1_BUFS, space="PSUM"))
        ps2_p = ctx.enter_context(tc.tile_pool(name="ps2", bufs=PS2_BUFS, space="PSUM"))

        br = const_p.tile([68, 128], BF16)
        bw = const_p.tile([128, 512], BF16)
        nc.sync.dma_start(br[:], BR_d)
        nc.sync.dma_start(bw[:], BW_d)

        sp0 = sp_p.tile([68, K, W], BF16)
        sp1 = sp_p.tile([68, K, W], BF16)
        # split loads so the first product octets can start sooner
        nc.sync.dma_start(sp0[:, 0:8, :], S_d[0:68, 0:8, :])
        nc.sync.dma_start(sp0[:, 8:16, :], S_d[0:68, 8:16, :])
        nc.sync.dma_start(sp1[:, 0:8, :], S_d[64:132, 0:8, :])
        nc.sync.dma_start(sp1[:, 8:16, :], S_d[64:132, 8:16, :])
        sps = [sp0, sp1]

        state = {}

        def stage1_block(rt, ocs):
            sp = sps[rt]
            brt = br[:, rt * 64:(rt + 1) * 64]
            if rt not in state:
                state[rt] = {"i1": i1_p.tile([128, 2, NCH, 64], BF16,
                                             name="i1"),
                             "mubs": [], "Ms": []}
            i1 = state[rt]["i1"]
            mubs = state[rt]["mubs"]
            Ms = state[rt]["Ms"]
            for oc in ocs:
                if oc < 2:
                    T = sp[:, oc * 8:(oc + 1) * 8, :]
                else:
                    Tt = t_p.tile([68, 8, W], BF16, name="T")
                    for (j0, k, l0, nl) in _ksegs_in_octet(oc):
                        in0 = sp[:, k, :].unsqueeze(1).broadcast_to([68, nl, W])
                        eng = nc.gpsimd if k < A_POOL_KMAX else nc.vector
                        eng.tensor_mul(
                            Tt[:, j0:j0 + nl, :], in0, sp[:, l0:l0 + nl, :])
                    T = Tt[:]
                ps1 = ps1_p.tile([128, 2, 8, 64], F32, name="ps1")
                for j in range(8):
                    nc.tensor.matmul(ps1[:, 0, j, :],
                                     T[0:68, j, 0:128], brt,
                                     start=True, stop=True)
                    nc.tensor.matmul(ps1[:, 1, j, :],
                                     T[0:68, j, 128:256], brt,
                                     start=True, stop=True)
                nc.scalar.copy(i1[:, :, oc * 8:(oc + 1) * 8, :], ps1[:])
                if oc == 1:
                    # hoist mean W-box + mu: Act/PE handle it while DVE is
                    # still busy with products, unblocking M-builds early
                    mubs = state[rt]["mubs"]
                    for oh in range(2):
                        bwa = bw[:, (oh * 2) * 128:(oh * 2) * 128 + 128]
                        bwb = bw[:, (oh * 2 + 1) * 128:(oh * 2 + 1) * 128 + 128]
                        mub = mu_p.tile([128, K, 64], BF16, name="mub")
                        mubs.append(mub)
                        for mo in range(2):
                            psm = ps2_p.tile([128, 8, 64], F32, name="ps2")
                            nc.tensor.matmul(psm[:], bwa,
                                             i1[:, 0, mo * 8:(mo + 1) * 8, :],
                                             start=True, stop=False)
                            nc.tensor.matmul(psm[:], bwb,
                                             i1[:, 1, mo * 8:(mo + 1) * 8, :],
                                             start=False, stop=True)
                            nc.scalar.mul(mub[:, mo * 8:(mo + 1) * 8, :],
                                          psm[:], 0.2)
                if oc in (4, 6):
                    # hoist M = mu_k*mu_l build between product octets
                    oh = 0 if oc == 4 else 1
                    mub = mubs[oh]
                    M = m_p.tile([128, NP, 64], BF16, name="M")
                    Ms.append(M)
                    for k in range(K):
                        nl = K - k
                        p0 = _pr0(k)
                        in0 = mub[:, k, :].unsqueeze(1).broadcast_to(
                            [128, nl, 64])
                        eng = nc.gpsimd if k < C_POOL_KMAX else nc.vector
                        eng.tensor_mul(M[:, p0:p0 + nl, :], in0,
                                       mub[:, k:K, :])

        def stage2_block(rt, oh):
                i1 = state[rt]["i1"]
                Ms = state[rt]["Ms"]
                bwa = bw[:, (oh * 2) * 128:(oh * 2) * 128 + 128]
                bwb = bw[:, (oh * 2 + 1) * 128:(oh * 2 + 1) * 128 + 128]
                M = Ms[oh]
                rsb = r_p.tile([128, NP, 64], BF16, name="rsb")
                for oc in range(2, NOCT):
                    ps2 = ps2_p.tile([128, 8, 64], F32, name="ps2")
                    nc.tensor.matmul(ps2[:], bwa,
                                     i1[:, 0, oc * 8:(oc + 1) * 8, :],
                                     start=True, stop=False)
                    nc.tensor.matmul(ps2[:], bwb,
                                     i1[:, 1, oc * 8:(oc + 1) * 8, :],
                                     start=False, stop=True)
                    pr = (oc - 2) * 8
                    dpf = D_POOL_FRAC[rt * 2 + oh]
                    gunit = (rt * 2 + oh) * 17 + (oc - 2)
                    if gunit % dpf == dpf - 1:
                        e2 = e2_p.tile([128, 8, 64], BF16, name="e2")
                        nc.scalar.copy(e2[:], ps2[:])
                        nc.gpsimd.tensor_sub(rsb[:, pr:pr + 8, :], e2[:],
                                             M[:, pr:pr + 8, :])
                    else:
                        nc.vector.tensor_sub(rsb[:, pr:pr + 8, :], ps2[:],
                                             M[:, pr:pr + 8, :])
                    # stream the store out in chunks so the final DMA only
                    # covers the tail octets instead of the whole triangle
                    if oc in DMA_SPLIT_AT:
                        lo = DMA_LO[oc]
                        nc.sync.dma_start(
                            R_d[oh * 128:(oh + 1) * 128, rt, lo:pr + 8, :],
                            rsb[:, lo:pr + 8, :])
                nc.sync.dma_start(
                    R_d[oh * 128:(oh + 1) * 128, rt, DMA_TAIL_LO:, :],
                    rsb[:, DMA_TAIL_LO:, :])

        stage1_block(0, range(NOCT))
        stage2_block(0, 0)
        stage1_block(1, range(0, S1_SPLIT))
        stage2_block(0, 1)
        stage1_block(1, range(S1_SPLIT, NOCT))
        stage2_block(1, 0)
        stage2_block(1, 1)

    nc.compile()
    return nc


_NC_CACHE = {}


def _get_nc():
    if "nc" not in _NC_CACHE:
        _NC_CACHE["nc"] = _build_kernel()
    return _NC_CACHE["nc"]


def _prep_in_maps(S):
    S = np.asarray(S, dtype=np.float32)
    np_bf16 = mybir.dt.np(BF16)
    bw = _build_bw().astype(np_bf16)
    brs = [(_build_br(h)).astype(np_bf16) for h in range(2)]
    Ss = S * np.float32(0.2)
    in_maps = []
    for b in range(B):
        for half in range(2):
            hbase = half * HH
            rows = np.clip(np.arange(hbase - 2, hbase + 130), 0, H - 1)
            shard = Ss[b][:, rows, :].transpose(1, 0, 2)
            shard = np.ascontiguousarray(shard).astype(np_bf16)
            in_maps.append({"S": shard, "BR": brs[half], "BW": bw})
    return in_maps


_IU = np.zeros((K, K), dtype=np.int64)
for _k in range(K):
    for _l in range(_k, K):
        _IU[_k, _l] = _IU[_l, _k] = _pr0(_k) + (_l - _k)


def _assemble(results):
    out = np.empty((B, H, W, K, K), dtype=np.float32)
    for i in range(8):
        b, half = divmod(i, 2)
        rd = np.asarray(results[i]["R"]).astype(np.float32)
        tri = rd.transpose(1, 3, 0, 2).reshape(HH, W, NP)
        out[b, half * HH:(half + 1) * HH] = tri[:, :, _IU]
    return out


def kernel(S):
    nc = _get_nc()
    in_maps = _prep_in_maps(S)
    res = bass_utils.run_bass_kernel_spmd(nc, in_maps, list(range(8)))
    return _assemble(res.results)


# revision 45
# speedup vs baseline: 1.4719x; 1.0179x over previous
"""2a' structure (per-rt phases) with engine-routing knobs.

R = box5x5(S_k*S_l)/25 - mu_k*mu_l; banded matmuls on TensorE; triangle
output; per-rt stage1/stage2 phases (rt-level overlap)."""
import sys

sys.path.insert(0, "/opt/trn_rl_repo")

from contextlib import ExitStack

import numpy as np

import concourse.bacc as bacc
import concourse.mybir as mybir
import concourse.tile as tile
from concourse import bass_utils

B, K, H, W = 4, 16, 256, 256
HH = 128
SR = 132
NP = 136
NCH = K + NP
NOCT = NCH // 8
F32 = mybir.dt.float32
BF16 = mybir.dt.bfloat16

D_POOL_FRAC = (4, 3, 3, 3)   # per (rt*2+oh) phase: every Nth sub on GpSimd
T_BUFS = 4
E2_BUFS = 4
M_BUFS = 3
PS1_BUFS = 2
PS2_BUFS = 4
C_POOL_KMAX = 7    # M k-runs with k < KMAX on GpSimd
A_POOL_KMAX = 0    # product k-runs with k < KMAX on GpSimd
S1_SPLIT = 0       # rt1 stage-1 octets emitted before stage2(rt0, oh1)
# stream output DMA: emit partial stores after these octets
DMA_SPLIT_AT = (4, 8, 12, 15, 17)
_splits = (2,) + tuple(o + 1 for o in DMA_SPLIT_AT)
DMA_LO = {o: (_splits[i] - 2) * 8 for i, o in enumerate(DMA_SPLIT_AT)}
DMA_TAIL_LO = (_splits[-1] - 2) * 8


def _reflect_idx(i, n):
    if i < 0:
        return -i
    if i >= n:
        return 2 * (n - 1) - i
    return i


def _build_bw():
    M = np.zeros((W, W), dtype=np.float32)
    for w in range(W):
        for j in range(5):
            M[_reflect_idx(w - 2 + j, W), w] += 1.0
    out = np.zeros((128, 512), dtype=np.float32)
    for oh in range(2):
        for c in range(2):
            out[:, (oh * 2 + c) * 128:(oh * 2 + c) * 128 + 128] = \
                M[c * 128:(c + 1) * 128, oh * 128:(oh + 1) * 128]
    return out


def _build_br(half):
    hbase = half * HH
    M = np.zeros((68, 128), dtype=np.float32)
    for rt in range(2):
        for hl in range(64):
            hg = hbase + rt * 64 + hl
            for i in range(5):
                r = _reflect_idx(hg - 2 + i, H)
                j = r + 2 - hbase
                M[j - rt * 64, rt * 64 + hl] += 1.0
    return M


def _ksegs_in_octet(oct_idx):
    lo, hi = oct_idx * 8, oct_idx * 8 + 8
    segs = []
    p = 0
    for k in range(K):
        n = K - k
        s, e = 16 + p, 16 + p + n
        a, b = max(lo, s), min(hi, e)
        if a < b:
            segs.append((a - lo, k, k + (a - s), b - a))
        p += n
    return segs


def _pr0(k):
    return k * K - (k * (k - 1)) // 2


def _build_kernel():
    nc = bacc.Bacc("TRN2", target_bir_lowering=False, debug=False)
    S_d = nc.dram_tensor("S", [SR, K, W], BF16, kind="ExternalInput").ap()
    BR_d = nc.dram_tensor("BR", [68, 128], BF16, kind="ExternalInput").ap()
    BW_d = nc.dram_tensor("BW", [128, 512], BF16, kind="ExternalInput").ap()
    R_d = nc.dram_tensor("R", [W, 2, NP, 64], BF16, kind="ExternalOutput").ap()

    with tile.TileContext(nc) as tc, ExitStack() as ctx:
        const_p = ctx.enter_context(tc.tile_pool(name="const", bufs=1))
        sp_p = ctx.enter_context(tc.tile_pool(name="sp", bufs=1))
        t_p = ctx.enter_context(tc.tile_pool(name="tprod", bufs=T_BUFS))
        i1_p = ctx.enter_context(tc.tile_pool(name="i1", bufs=2))
        mu_p = ctx.enter_context(tc.tile_pool(name="mu", bufs=2))
        m_p = ctx.enter_context(tc.tile_pool(name="mm", bufs=M_BUFS))
        r_p = ctx.enter_context(tc.tile_pool(name="rout", bufs=2))
        e2_p = ctx.enter_context(tc.tile_pool(name="e2", bufs=E2_BUFS))
        ps1_p = ctx.enter_context(tc.tile_pool(name="ps1", bufs=PS1_BUFS, space="PSUM"))
        ps2_p = ctx.enter_context(tc.tile_pool(name="ps2", bufs=PS2_BUFS, space="PSUM"))

        br = const_p.tile([68, 128], BF16)
        bw = const_p.tile([128, 512], BF16)
        sp0 = sp_p.tile([68, K, W], BF16)
        sp1 = sp_p.tile([68, K, W], BF16)
        # split loads so the first product octets can start sooner
        nc.sync.dma_start(sp0[:, 0:8, :], S_d[0:68, 0:8, :])
        nc.sync.dma_start(sp0[:, 8:16, :], S_d[0:68, 8:16, :])
        nc.sync.dma_start(br[:], BR_d)
        nc.sync.dma_start(bw[:], BW_d)
        nc.sync.dma_start(sp1[:, 0:8, :], S_d[64:132, 0:8, :])
        nc.sync.dma_start(sp1[:, 8:16, :], S_d[64:132, 8:16, :])
        sps = [sp0, sp1]

        state = {}

        def stage1_block(rt, ocs):
            sp = sps[rt]
            brt = br[:, rt * 64:(rt + 1) * 64]
            if rt not in state:
                state[rt] = {"i1": i1_p.tile([128, 2, NCH, 64], BF16,
                                             name="i1"),
                             "mubs": [], "Ms": []}
            i1 = state[rt]["i1"]
            mubs = state[rt]["mubs"]
            Ms = state[rt]["Ms"]
            for oc in ocs:
                if oc < 2:
                    T = sp[:, oc * 8:(oc + 1) * 8, :]
                else:
                    Tt = t_p.tile([68, 8, W], BF16, name="T")
                    for (j0, k, l0, nl) in _ksegs_in_octet(oc):
                        in0 = sp[:, k, :].unsqueeze(1).broadcast_to([68, nl, W])
                        eng = nc.gpsimd if k < A_POOL_KMAX else nc.vector
                        eng.tensor_mul(
                            Tt[:, j0:j0 + nl, :], in0, sp[:, l0:l0 + nl, :])
                    T = Tt[:]
                ps1 = ps1_p.tile([128, 2, 8, 64], F32, name="ps1")
                for j in range(8):
                    nc.tensor.matmul(ps1[:, 0, j, :],
                                     T[0:68, j, 0:128], brt,
                                     start=True, stop=True)
                    nc.tensor.matmul(ps1[:, 1, j, :],
                                     T[0:68, j, 128:256], brt,
                                     start=True, stop=True)
                nc.scalar.copy(i1[:, :, oc * 8:(oc + 1) * 8, :], ps1[:])
                if oc == 1:
                    # hoist mean W-box + mu: Act/PE handle it while DVE is
                    # still busy with products, unblocking M-builds early
                    mubs = state[rt]["mubs"]
                    for oh in range(2):
                        bwa = bw[:, (oh * 2) * 128:(oh * 2) * 128 + 128]
                        bwb = bw[:, (oh * 2 + 1) * 128:(oh * 2 + 1) * 128 + 128]
                        mub = mu_p.tile([128, K, 64], BF16, name="mub")
                        mubs.append(mub)
                        for mo in range(2):
                            psm = ps2_p.tile([128, 8, 64], F32, name="ps2")
                            nc.tensor.matmul(psm[:], bwa,
                                             i1[:, 0, mo * 8:(mo + 1) * 8, :],
                                             start=True, stop=False)
                            nc.tensor.matmul(psm[:], bwb,
                                             i1[:, 1, mo * 8:(mo + 1) * 8, :],
                                             start=False, stop=True)
                            nc.scalar.mul(mub[:, mo * 8:(mo + 1) * 8, :],
                                          psm[:], 0.2)
                if oc in (4, 6):
                    # hoist M = mu_k*mu_l build between product octets
                    oh = 0 if oc == 4 else 1
                    mub = mubs[oh]
                    M = m_p.tile([128, NP, 64], BF16, name="M")
                    Ms.append(M)
                    for k in range(K):
                        nl = K - k
                        p0 = _pr0(k)
                        in0 = mub[:, k, :].unsqueeze(1).broadcast_to(
                            [128, nl, 64])
                        eng = nc.gpsimd if k < C_POOL_KMAX else nc.vector
                        eng.tensor_mul(M[:, p0:p0 + nl, :], in0,
                                       mub[:, k:K, :])

        def stage2_block(rt, oh):
                i1 = state[rt]["i1"]
                Ms = state[rt]["Ms"]
                bwa = bw[:, (oh * 2) * 128:(oh * 2) * 128 + 128]
                bwb = bw[:, (oh * 2 + 1) * 128:(oh * 2 + 1) * 128 + 128]
                M = Ms[oh]
                rsb = r_p.tile([128, NP, 64], BF16, name="rsb")
                for oc in range(2, NOCT):
                    ps2 = ps2_p.tile([128, 8, 64], F32, name="ps2")
                    nc.tensor.matmul(ps2[:], bwa,
                                     i1[:, 0, oc * 8:(oc + 1) * 8, :],
                                     start=True, stop=False)
                    nc.tensor.matmul(ps2[:], bwb,
                                     i1[:, 1, oc * 8:(oc + 1) * 8, :],
                                     start=False, stop=True)
                    pr = (oc - 2) * 8
                    dpf = D_POOL_FRAC[rt * 2 + oh]
                    gunit = (rt * 2 + oh) * 17 + (oc - 2)
                    if gunit % dpf == dpf - 1:
                        e2 = e2_p.tile([128, 8, 64], BF16, name="e2")
                        nc.scalar.copy(e2[:], ps2[:])
                        nc.gpsimd.tensor_sub(rsb[:, pr:pr + 8, :], e2[:],
                                             M[:, pr:pr + 8, :])
                    else:
                        nc.vector.tensor_sub(rsb[:, pr:pr + 8, :], ps2[:],
                                             M[:, pr:pr + 8, :])
                    # stream the store out in chunks so the final DMA only
                    # covers the tail octets instead of the whole triangle
                    if oc in DMA_SPLIT_AT:
                        lo = DMA_LO[oc]
                        nc.sync.dma_start(
                            R_d[oh * 128:(oh + 1) * 128, rt, lo:pr + 8, :],
                            rsb[:, lo:pr + 8, :])
                nc.sync.dma_start(
                    R_d[oh * 128:(oh + 1) * 128, rt, DMA_TAIL_LO:, :],
                    rsb[:, DMA_TAIL_LO:, :])

        stage1_block(0, range(NOCT))
        stage2_block(0, 0)
        stage1_block(1, range(0, S1_SPLIT))
        stage2_block(0, 1)
        stage1_block(1, range(S1_SPLIT, NOCT))
        stage2_block(1, 0)
        stage2_block(1, 1)

    nc.compile()
    return nc


_NC_CACHE = {}


def _get_nc():
    if "nc" not in _NC_CACHE:
        _NC_CACHE["nc"] = _build_kernel()
    return _NC_CACHE["nc"]


def _prep_in_maps(S):
    S = np.asarray(S, dtype=np.float32)
    np_bf16 = mybir.dt.np(BF16)
    bw = _build_bw().astype(np_bf16)
    brs = [(_build_br(h)).astype(np_bf16) for h in range(2)]
    Ss = S * np.float32(0.2)
    in_maps = []
    for b in range(B):
        for half in range(2):
            hbase = half * HH
            rows = np.clip(np.arange(hbase - 2, hbase + 130), 0, H - 1)
            shard = Ss[b][:, rows, :].transpose(1, 0, 2)
            shard = np.ascontiguousarray(shard).astype(np_bf16)
            in_maps.append({"S": shard, "BR": brs[half], "BW": bw})
    return in_maps


_IU = np.zeros((K, K), dtype=np.int64)
for _k in range(K):
    for _l in range(_k, K):
        _IU[_k, _l] = _IU[_l, _k] = _pr0(_k) + (_l - _k)


def _assemble(results):
    out = np.empty((B, H, W, K, K), dtype=np.float32)
    for i in range(8):
        b, half = divmod(i, 2)
        rd = np.asarray(results[i]["R"]).astype(np.float32)
        tri = rd.transpose(1, 3, 0, 2).reshape(HH, W, NP)
        out[b, half * HH:(half + 1) * HH] = tri[:, :, _IU]
    return out


def kernel(S):
    nc = _get_nc()
    in_maps = _prep_in_maps(S)
    res = bass_utils.run_bass_kernel_spmd(nc, in_maps, list(range(8)))
    return _assemble(res.results)


# revision 46
# speedup vs baseline: 1.4815x; 1.0066x over previous
"""2a' structure (per-rt phases) with engine-routing knobs.

R = box5x5(S_k*S_l)/25 - mu_k*mu_l; banded matmuls on TensorE; triangle
output; per-rt stage1/stage2 phases (rt-level overlap)."""
import sys

sys.path.insert(0, "/opt/trn_rl_repo")

from contextlib import ExitStack

import numpy as np

import concourse.bacc as bacc
import concourse.mybir as mybir
import concourse.tile as tile
from concourse import bass_utils

B, K, H, W = 4, 16, 256, 256
HH = 128
SR = 132
NP = 136
NCH = K + NP
NOCT = NCH // 8
F32 = mybir.dt.float32
BF16 = mybir.dt.bfloat16

D_POOL_FRAC = (4, 3, 3, 3)   # per (rt*2+oh) phase: every Nth sub on GpSimd
T_BUFS = 5
E2_BUFS = 4
M_BUFS = 3
PS1_BUFS = 2
PS2_BUFS = 4
C_POOL_KMAX = 7    # M k-runs with k < KMAX on GpSimd
A_POOL_KMAX = 0    # product k-runs with k < KMAX on GpSimd
S1_SPLIT = 0       # rt1 stage-1 octets emitted before stage2(rt0, oh1)
# stream output DMA: emit partial stores after these octets
DMA_SPLIT_AT = (4, 8, 12, 15, 17)
_splits = (2,) + tuple(o + 1 for o in DMA_SPLIT_AT)
DMA_LO = {o: (_splits[i] - 2) * 8 for i, o in enumerate(DMA_SPLIT_AT)}
DMA_TAIL_LO = (_splits[-1] - 2) * 8


def _reflect_idx(i, n):
    if i < 0:
        return -i
    if i >= n:
        return 2 * (n - 1) - i
    return i


def _build_bw():
    M = np.zeros((W, W), dtype=np.float32)
    for w in range(W):
        for j in range(5):
            M[_reflect_idx(w - 2 + j, W), w] += 1.0
    out = np.zeros((128, 512), dtype=np.float32)
    for oh in range(2):
        for c in range(2):
            out[:, (oh * 2 + c) * 128:(oh * 2 + c) * 128 + 128] = \
                M[c * 128:(c + 1) * 128, oh * 128:(oh + 1) * 128]
    return out


def _build_br(half):
    hbase = half * HH
    M = np.zeros((68, 128), dtype=np.float32)
    for rt in range(2):
        for hl in range(64):
            hg = hbase + rt * 64 + hl
            for i in range(5):
                r = _reflect_idx(hg - 2 + i, H)
                j = r + 2 - hbase
                M[j - rt * 64, rt * 64 + hl] += 1.0
    return M


def _ksegs_in_octet(oct_idx):
    lo, hi = oct_idx * 8, oct_idx * 8 + 8
    segs = []
    p = 0
    for k in range(K):
        n = K - k
        s, e = 16 + p, 16 + p + n
        a, b = max(lo, s), min(hi, e)
        if a < b:
            segs.append((a - lo, k, k + (a - s), b - a))
        p += n
    return segs


def _pr0(k):
    return k * K - (k * (k - 1)) // 2


def _build_kernel():
    nc = bacc.Bacc("TRN2", target_bir_lowering=False, debug=False)
    S_d = nc.dram_tensor("S", [SR, K, W], BF16, kind="ExternalInput").ap()
    BR_d = nc.dram_tensor("BR", [68, 128], BF16, kind="ExternalInput").ap()
    BW_d = nc.dram_tensor("BW", [128, 512], BF16, kind="ExternalInput").ap()
    R_d = nc.dram_tensor("R", [W, 2, NP, 64], BF16, kind="ExternalOutput").ap()

    with tile.TileContext(nc) as tc, ExitStack() as ctx:
        const_p = ctx.enter_context(tc.tile_pool(name="const", bufs=1))
        sp_p = ctx.enter_context(tc.tile_pool(name="sp", bufs=1))
        t_p = ctx.enter_context(tc.tile_pool(name="tprod", bufs=T_BUFS))
        i1_p = ctx.enter_context(tc.tile_pool(name="i1", bufs=2))
        mu_p = ctx.enter_context(tc.tile_pool(name="mu", bufs=2))
        m_p = ctx.enter_context(tc.tile_pool(name="mm", bufs=M_BUFS))
        r_p = ctx.enter_context(tc.tile_pool(name="rout", bufs=2))
        e2_p = ctx.enter_context(tc.tile_pool(name="e2", bufs=E2_BUFS))
        ps1_p = ctx.enter_context(tc.tile_pool(name="ps1", bufs=PS1_BUFS, space="PSUM"))
        ps2_p = ctx.enter_context(tc.tile_pool(name="ps2", bufs=PS2_BUFS, space="PSUM"))

        br = const_p.tile([68, 128], BF16)
        bw = const_p.tile([128, 512], BF16)
        sp0 = sp_p.tile([68, K, W], BF16)
        sp1 = sp_p.tile([68, K, W], BF16)
        # split loads so the first product octets can start sooner
        nc.sync.dma_start(sp0[:, 0:8, :], S_d[0:68, 0:8, :])
        nc.sync.dma_start(sp0[:, 8:16, :], S_d[0:68, 8:16, :])
        nc.sync.dma_start(br[:], BR_d)
        nc.sync.dma_start(bw[:], BW_d)
        nc.sync.dma_start(sp1[:, 0:8, :], S_d[64:132, 0:8, :])
        nc.sync.dma_start(sp1[:, 8:16, :], S_d[64:132, 8:16, :])
        sps = [sp0, sp1]

        state = {}

        def stage1_block(rt, ocs):
            sp = sps[rt]
            brt = br[:, rt * 64:(rt + 1) * 64]
            if rt not in state:
                state[rt] = {"i1": i1_p.tile([128, 2, NCH, 64], BF16,
                                             name="i1"),
                             "mubs": [], "Ms": []}
            i1 = state[rt]["i1"]
            mubs = state[rt]["mubs"]
            Ms = state[rt]["Ms"]
            for oc in ocs:
                if oc < 2:
                    T = sp[:, oc * 8:(oc + 1) * 8, :]
                else:
                    Tt = t_p.tile([68, 8, W], BF16, name="T")
                    for (j0, k, l0, nl) in _ksegs_in_octet(oc):
                        in0 = sp[:, k, :].unsqueeze(1).broadcast_to([68, nl, W])
                        eng = nc.gpsimd if k < A_POOL_KMAX else nc.vector
                        eng.tensor_mul(
                            Tt[:, j0:j0 + nl, :], in0, sp[:, l0:l0 + nl, :])
                    T = Tt[:]
                ps1 = ps1_p.tile([128, 2, 8, 64], F32, name="ps1")
                for j in range(8):
                    nc.tensor.matmul(ps1[:, 0, j, :],
                                     T[0:68, j, 0:128], brt,
                                     start=True, stop=True)
                    nc.tensor.matmul(ps1[:, 1, j, :],
                                     T[0:68, j, 128:256], brt,
                                     start=True, stop=True)
                nc.scalar.copy(i1[:, :, oc * 8:(oc + 1) * 8, :], ps1[:])
                if oc == 1:
                    # hoist mean W-box + mu: Act/PE handle it while DVE is
                    # still busy with products, unblocking M-builds early
                    mubs = state[rt]["mubs"]
                    for oh in range(2):
                        bwa = bw[:, (oh * 2) * 128:(oh * 2) * 128 + 128]
                        bwb = bw[:, (oh * 2 + 1) * 128:(oh * 2 + 1) * 128 + 128]
                        mub = mu_p.tile([128, K, 64], BF16, name="mub")
                        mubs.append(mub)
                        for mo in range(2):
                            psm = ps2_p.tile([128, 8, 64], F32, name="ps2")
                            nc.tensor.matmul(psm[:], bwa,
                                             i1[:, 0, mo * 8:(mo + 1) * 8, :],
                                             start=True, stop=False)
                            nc.tensor.matmul(psm[:], bwb,
                                             i1[:, 1, mo * 8:(mo + 1) * 8, :],
                                             start=False, stop=True)
                            nc.scalar.mul(mub[:, mo * 8:(mo + 1) * 8, :],
                                          psm[:], 0.2)
                if oc in (4, 6):
                    # hoist M = mu_k*mu_l build between product octets
                    oh = 0 if oc == 4 else 1
                    mub = mubs[oh]
                    M = m_p.tile([128, NP, 64], BF16, name="M")
                    Ms.append(M)
                    for k in range(K):
                        nl = K - k
                        p0 = _pr0(k)
                        in0 = mub[:, k, :].unsqueeze(1).broadcast_to(
                            [128, nl, 64])
                        eng = nc.gpsimd if k < C_POOL_KMAX else nc.vector
                        eng.tensor_mul(M[:, p0:p0 + nl, :], in0,
                                       mub[:, k:K, :])

        def stage2_block(rt, oh):
                i1 = state[rt]["i1"]
                Ms = state[rt]["Ms"]
                bwa = bw[:, (oh * 2) * 128:(oh * 2) * 128 + 128]
                bwb = bw[:, (oh * 2 + 1) * 128:(oh * 2 + 1) * 128 + 128]
                M = Ms[oh]
                rsb = r_p.tile([128, NP, 64], BF16, name="rsb")
                for oc in range(2, NOCT):
                    ps2 = ps2_p.tile([128, 8, 64], F32, name="ps2")
                    nc.tensor.matmul(ps2[:], bwa,
                                     i1[:, 0, oc * 8:(oc + 1) * 8, :],
                                     start=True, stop=False)
                    nc.tensor.matmul(ps2[:], bwb,
                                     i1[:, 1, oc * 8:(oc + 1) * 8, :],
                                     start=False, stop=True)
                    pr = (oc - 2) * 8
                    dpf = D_POOL_FRAC[rt * 2 + oh]
                    gunit = (rt * 2 + oh) * 17 + (oc - 2)
                    if gunit % dpf == dpf - 1:
                        e2 = e2_p.tile([128, 8, 64], BF16, name="e2")
                        nc.scalar.copy(e2[:], ps2[:])
                        nc.gpsimd.tensor_sub(rsb[:, pr:pr + 8, :], e2[:],
                                             M[:, pr:pr + 8, :])
                    else:
                        nc.vector.tensor_sub(rsb[:, pr:pr + 8, :], ps2[:],
                                             M[:, pr:pr + 8, :])
                    # stream the store out in chunks so the final DMA only
                    # covers the tail octets instead of the whole triangle
                    if oc in DMA_SPLIT_AT:
                        lo = DMA_LO[oc]
                        nc.sync.dma_start(
                            R_d[oh * 128:(oh + 1) * 128, rt, lo:pr + 8, :],
                            rsb[:, lo:pr + 8, :])
                nc.sync.dma_start(
                    R_d[oh * 128:(oh + 1) * 128, rt, DMA_TAIL_LO:, :],
                    rsb[:, DMA_TAIL_LO:, :])

        stage1_block(0, range(NOCT))
        stage2_block(0, 0)
        stage1_block(1, range(0, S1_SPLIT))
        stage2_block(0, 1)
        stage1_block(1, range(S1_SPLIT, NOCT))
        stage2_block(1, 0)
        stage2_block(1, 1)

    nc.compile()
    return nc


_NC_CACHE = {}


def _get_nc():
    if "nc" not in _NC_CACHE:
        _NC_CACHE["nc"] = _build_kernel()
    return _NC_CACHE["nc"]


def _prep_in_maps(S):
    S = np.asarray(S, dtype=np.float32)
    np_bf16 = mybir.dt.np(BF16)
    bw = _build_bw().astype(np_bf16)
    brs = [(_build_br(h)).astype(np_bf16) for h in range(2)]
    Ss = S * np.float32(0.2)
    in_maps = []
    for b in range(B):
        for half in range(2):
            hbase = half * HH
            rows = np.clip(np.arange(hbase - 2, hbase + 130), 0, H - 1)
            shard = Ss[b][:, rows, :].transpose(1, 0, 2)
            shard = np.ascontiguousarray(shard).astype(np_bf16)
            in_maps.append({"S": shard, "BR": brs[half], "BW": bw})
    return in_maps


_IU = np.zeros((K, K), dtype=np.int64)
for _k in range(K):
    for _l in range(_k, K):
        _IU[_k, _l] = _IU[_l, _k] = _pr0(_k) + (_l - _k)


def _assemble(results):
    out = np.empty((B, H, W, K, K), dtype=np.float32)
    for i in range(8):
        b, half = divmod(i, 2)
        rd = np.asarray(results[i]["R"]).astype(np.float32)
        tri = rd.transpose(1, 3, 0, 2).reshape(HH, W, NP)
        out[b, half * HH:(half + 1) * HH] = tri[:, :, _IU]
    return out


def kernel(S):
    nc = _get_nc()
    in_maps = _prep_in_maps(S)
    res = bass_utils.run_bass_kernel_spmd(nc, in_maps, list(range(8)))
    return _assemble(res.results)
